# revision 1
# baseline (speedup 1.0000x reference)
"""KAN-GNN message passing on 8 TRN2 NeuronCores.

Strategy (data-parallel over nodes, per sharding hint):
 - Nodes are ranked by in-degree and dealt round-robin to the 8 cores, so
   every core holds 12500 targets with a near-identical degree profile.
 - Per core: KAN layer 1 on its node shard (3 fused matmuls: x, x^2, x^3
   against host-combined weights, bias via PSUM pre-init, relu+pad-mask in
   the ACT drain), then an AllGather of the bf16 h1 table.
 - Aggregation = one indirect-DMA gather per 128-target group: each target
   (partition) pulls its padded list of source rows side by side in the
   free dim, then a contiguous halving tree of DVE adds reduces the slots;
   scale by 1/deg, PE-transpose to put features on partitions, KAN layer 2,
   AllGather of h2, second gather/reduce, log_softmax.
 - All indices/permutations are precomputed on the host and baked into the
   (single, SPMD) program; per-core data goes in as input tensors.
"""
import numpy as np
import ml_dtypes

import concourse.bacc as bacc
import concourse.mybir as mybir
import concourse.tile as tile
import concourse.bass as bass
from concourse.bass_utils import run_bass_kernel_spmd

N_NODES = 100000
N_EDGES = 1600000
IN_F, HID_F, OUT_F = 128, 128, 64
K = 8               # cores
P = 128             # partitions / targets per group
J = 12544           # local nodes per core (98 * 128), 12500 real + 44 pad
G = J // P          # 98 groups
JREAL = N_NODES // K  # 12500
TBL = K * J         # 100352 rows in the all-gathered tables
PAD_POS = JREAL     # position (core 0, j=12500) -> guaranteed zero row

BF16 = mybir.dt.bfloat16
F32 = mybir.dt.float32
I32 = mybir.dt.int32


def _host_prep(x, edge_index, w1, b1, c1, w2, b2, c2):
    src = np.asarray(edge_index[0], dtype=np.int64)
    tgt = np.asarray(edge_index[1], dtype=np.int64)
    x = np.asarray(x, dtype=np.float32)

    deg = np.bincount(tgt, minlength=N_NODES)
    order = np.argsort(-deg, kind="stable")          # global degree rank -> node
    rank_of = np.empty(N_NODES, dtype=np.int64)
    rank_of[order] = np.arange(N_NODES)
    core_of_node = rank_of % K
    j_of_node = rank_of // K
    pos_of_node = core_of_node * J + j_of_node        # row in AG tables

    # per-core local degree [K, J]
    degs_kj = np.zeros((K, J), dtype=np.int64)
    degs_kj[core_of_node, j_of_node] = deg

    # group slot counts (shared across cores -> same program)
    Dg = degs_kj.reshape(K, G, P).max(axis=(0, 2))    # [G]
    Dg = np.maximum(Dg, 1).astype(np.int64)
    offs = np.concatenate([[0], np.cumsum(Dg)])       # [G+1]
    S = int(offs[-1])

    # slot table: idx_all[k, p, col] = table position of the d-th source of
    # local target (g*128+p) on core k; PAD_POS when d >= degree.
    idx_all = np.full((K, P, S), PAD_POS, dtype=np.int32)
    ek = core_of_node[tgt]
    ej = j_of_node[tgt]
    key = ek * J + ej
    eorder = np.argsort(key, kind="stable")
    skey = key[eorder]
    ukey, counts = np.unique(skey, return_counts=True)
    run_starts = np.concatenate([[0], np.cumsum(counts)[:-1]])
    d_in_run = np.arange(len(skey)) - np.repeat(run_starts, counts)
    ks = skey // J
    js = skey % J
    gs = js // P
    ps = js % P
    cols = offs[gs] + d_in_run
    idx_all[ks, ps, cols] = pos_of_node[src[eorder]].astype(np.int32)

    # per-core 1/deg  [K, P, G]  (0 for pad targets)
    with np.errstate(divide="ignore"):
        dr = 1.0 / np.maximum(degs_kj, 1).astype(np.float32)
    real = np.zeros((K, J), dtype=np.float32)
    real[:, :JREAL] = 1.0
    # reference divides by max(deg,1); deg-0 real targets get sum 0 -> 0 fine
    degrecip = (dr * np.where(real > 0, 1.0, 0.0)).reshape(K, G, P).transpose(0, 2, 1).copy()

    # pad-node mask [P, G] (same on every core)
    mask_j = (np.arange(J) < JREAL).astype(np.float32)
    mask_pg = mask_j.reshape(G, P).T.copy()

    # xT shards, bf16 [K][IN_F, J]
    xT = np.zeros((K, IN_F, J), dtype=ml_dtypes.bfloat16)
    for k in range(K):
        nodes_k = order[np.arange(JREAL) * K + k]
        xT[k, :, :JREAL] = x[nodes_k].T.astype(ml_dtypes.bfloat16)

    # fused KAN weights
    A1 = (w1 + 0.1 * c1[:, :, 0]).astype(ml_dtypes.bfloat16)
    B1 = (0.1 * c1[:, :, 1]).astype(ml_dtypes.bfloat16)
    C1 = (0.1 * c1[:, :, 2]).astype(ml_dtypes.bfloat16)
    A2 = (w2 + 0.1 * c2[:, :, 0]).astype(ml_dtypes.bfloat16)
    B2 = (0.1 * c2[:, :, 1]).astype(ml_dtypes.bfloat16)
    C2 = (0.1 * c2[:, :, 2]).astype(ml_dtypes.bfloat16)
    b1b = np.tile(np.asarray(b1, np.float32)[None, :], (P, 1))
    b2b = np.tile(np.asarray(b2, np.float32)[None, :], (P, 1))
    ident = np.eye(P, dtype=np.float32)

    in_maps = []
    for k in range(K):
        in_maps.append({
            "xT": xT[k],
            "idx": idx_all[k],
            "degrecip": degrecip[k],
            "mask": mask_pg,
            "A1": A1, "B1": B1, "C1": C1,
            "A2": A2, "B2": B2, "C2": C2,
            "b1b": b1b, "b2b": b2b, "ident": ident,
        })
    meta = {"Dg": Dg, "offs": offs, "S": S, "order": order}
    return in_maps, meta


def _tree_reduce(nc, tiles_ap, D, F):
    """In-place halving tree over D slots of width F. Returns slice [P, F]."""
    cur = D
    while cur > 1:
        h = cur // 2
        nc.vector.tensor_tensor(
            out=tiles_ap[:, : h * F],
            in0=tiles_ap[:, : h * F],
            in1=tiles_ap[:, (cur - h) * F: cur * F],
            op=mybir.AluOpType.add,
        )
        cur = cur - h
    return tiles_ap[:, :F]


QNAMES = ["qPoolDynamic", "qPoolDynamic1", "qPoolDynamic2", "qPoolDynamic3"]


def build_program(Dg, offs, S, dump=False, reps=1, skip_coll=False, skip_gather=False):
    nc = bacc.Bacc("TRN2", target_bir_lowering=False, debug=False, num_devices=K,
                   dynamic_dma_scratch_size=131072, num_swdge_queues=4)

    xT = nc.dram_tensor("xT", [IN_F, J], BF16, kind="ExternalInput")
    idx = nc.dram_tensor("idx", [P, S], I32, kind="ExternalInput")
    degrecip = nc.dram_tensor("degrecip", [P, G], F32, kind="ExternalInput")
    mask = nc.dram_tensor("mask", [P, G], F32, kind="ExternalInput")
    A1 = nc.dram_tensor("A1", [IN_F, HID_F], BF16, kind="ExternalInput")
    B1 = nc.dram_tensor("B1", [IN_F, HID_F], BF16, kind="ExternalInput")
    C1 = nc.dram_tensor("C1", [IN_F, HID_F], BF16, kind="ExternalInput")
    A2 = nc.dram_tensor("A2", [HID_F, OUT_F], BF16, kind="ExternalInput")
    B2 = nc.dram_tensor("B2", [HID_F, OUT_F], BF16, kind="ExternalInput")
    C2 = nc.dram_tensor("C2", [HID_F, OUT_F], BF16, kind="ExternalInput")
    b1b = nc.dram_tensor("b1b", [P, HID_F], F32, kind="ExternalInput")
    b2b = nc.dram_tensor("b2b", [P, OUT_F], F32, kind="ExternalInput")
    ident = nc.dram_tensor("ident", [P, P], F32, kind="ExternalInput")
    y = nc.dram_tensor("y", [J, OUT_F], F32, kind="ExternalOutput")
    if dump:
        h1o = nc.dram_tensor("h1o", [J, HID_F], BF16, kind="ExternalOutput")
        sno = nc.dram_tensor("sno", [J, HID_F], F32, kind="ExternalOutput")

    h1_in = nc.dram_tensor("h1_in", [J, HID_F], BF16, kind="Internal")
    h1_tbl = nc.dram_tensor("h1_tbl", [TBL, HID_F], BF16, kind="Internal",
                            addr_space="Shared")
    h2_in = nc.dram_tensor("h2_in", [J, OUT_F], BF16, kind="Internal")
    h2_tbl = nc.dram_tensor("h2_tbl", [TBL, OUT_F], BF16, kind="Internal",
                            addr_space="Shared")

    Dmax = int(max(Dg))

    with tile.TileContext(nc) as tc:
        with (
            tc.tile_pool(name="consts", bufs=1) as cpool,
            tc.tile_pool(name="work", bufs=3) as wpool,
            tc.tile_pool(name="slots", bufs=5) as spool,
            tc.tile_pool(name="psum", bufs=2, space="PSUM") as ppool,
        ):
            # load constants
            c_idx = cpool.tile([P, S], I32, tag="idx")
            nc.sync.dma_start(out=c_idx[:], in_=idx[:, :])
            c_dr = cpool.tile([P, G], F32, tag="dr")
            nc.sync.dma_start(out=c_dr[:], in_=degrecip[:, :])
            c_mask = cpool.tile([P, G], F32, tag="mask")
            nc.sync.dma_start(out=c_mask[:], in_=mask[:, :])
            c_w1 = []
            for nm, t in (("A1", A1), ("B1", B1), ("C1", C1)):
                w = cpool.tile([IN_F, HID_F], BF16, tag=nm)
                nc.sync.dma_start(out=w[:], in_=t[:, :])
                c_w1.append(w)
            c_w2 = []
            for nm, t in (("A2", A2), ("B2", B2), ("C2", C2)):
                w = cpool.tile([HID_F, OUT_F], BF16, tag=nm)
                nc.sync.dma_start(out=w[:], in_=t[:, :])
                c_w2.append(w)
            c_b1 = cpool.tile([P, HID_F], F32, tag="b1b")
            nc.sync.dma_start(out=c_b1[:], in_=b1b[:, :])
            c_b2 = cpool.tile([P, OUT_F], F32, tag="b2b")
            nc.sync.dma_start(out=c_b2[:], in_=b2b[:, :])
            c_id = cpool.tile([P, P], F32, tag="ident")
            nc.sync.dma_start(out=c_id[:], in_=ident[:, :])

            for _rep in range(reps):
                # ---------------- phase 1: KAN layer 1 on the shard ----------------
                for g in range(G):
                    xt = wpool.tile([IN_F, P], BF16, tag="xt")
                    nc.sync.dma_start(out=xt[:], in_=xT[:, g * P:(g + 1) * P])
                    x2 = wpool.tile([IN_F, P], BF16, tag="x2")
                    nc.vector.tensor_tensor(out=x2[:], in0=xt[:], in1=xt[:],
                                            op=mybir.AluOpType.mult)
                    x3 = wpool.tile([IN_F, P], BF16, tag="x3")
                    nc.vector.tensor_tensor(out=x3[:], in0=x2[:], in1=xt[:],
                                            op=mybir.AluOpType.mult)
                    ps = ppool.tile([P, HID_F], F32, tag="k1")
                    nc.tensor.matmul(out=ps[:], lhsT=xt[:], rhs=c_w1[0][:],
                                     start=True, stop=False)
                    nc.tensor.matmul(out=ps[:], lhsT=x2[:], rhs=c_w1[1][:],
                                     start=False, stop=False)
                    nc.tensor.matmul(out=ps[:], lhsT=x3[:], rhs=c_w1[2][:],
                                     start=False, stop=True)
                    hb = wpool.tile([P, HID_F], F32, tag="hb")
                    nc.vector.tensor_tensor(out=hb[:], in0=ps[:], in1=c_b1[:],
                                            op=mybir.AluOpType.add)
                    h1t = wpool.tile([P, HID_F], BF16, tag="h1t")
                    nc.scalar.activation(out=h1t[:], in_=hb[:],
                                         func=mybir.ActivationFunctionType.Relu,
                                         scale=c_mask[:, g:g + 1])
                    nc.sync.dma_start(out=h1_in[g * P:(g + 1) * P, :], in_=h1t[:])
                    if dump:
                        nc.sync.dma_start(out=h1o[g * P:(g + 1) * P, :], in_=h1t[:])

                # ---------------- AllGather h1 ----------------
                if not skip_coll:
                    nc.gpsimd.collective_compute(
                        "AllGather", mybir.AluOpType.bypass,
                        replica_groups=[list(range(K))],
                        ins=[h1_in[:, :]], outs=[h1_tbl[:, :]],
                    )

                # ---------------- phase 2: aggregate + KAN layer 2 ----------------
                for g in range(G):
                    D = int(Dg[g])
                    off = int(offs[g])
                    sts = []
                    for d in range(D if not skip_gather else 0):
                        st = spool.tile([P, HID_F], BF16, tag=f"sl{d}")
                        inst = nc.gpsimd.indirect_dma_start(
                            out=st[:, :],
                            out_offset=None,
                            in_=h1_tbl[:, :],
                            in_offset=bass.IndirectOffsetOnAxis(
                                ap=c_idx[:, off + d:off + d + 1], axis=0),
                        )
                        inst.ins.queue = QNAMES[(off + d) % 4]
                        sts.append(st)
                    sn = wpool.tile([P, HID_F], F32, tag="sn")
                    if not sts:
                        nc.vector.memset(sn[:], 0.0)
                    elif len(sts) == 1:
                        nc.scalar.copy(out=sn[:], in_=sts[0][:])
                    else:
                        nc.vector.tensor_tensor(out=sn[:], in0=sts[0][:],
                                                in1=sts[1][:],
                                                op=mybir.AluOpType.add)
                        for d in range(2, len(sts)):
                            nc.vector.tensor_tensor(out=sn[:], in0=sn[:],
                                                    in1=sts[d][:],
                                                    op=mybir.AluOpType.add)
                    nc.vector.tensor_scalar_mul(sn[:], sn[:], c_dr[:, g:g + 1])
                    if dump:
                        nc.sync.dma_start(out=sno[g * P:(g + 1) * P, :], in_=sn[:])
                    pt = ppool.tile([P, P], F32, tag="tr")
                    nc.tensor.transpose(out=pt[:], in_=sn[:], identity=c_id[:])
                    hT = wpool.tile([HID_F, P], BF16, tag="hT")
                    nc.scalar.copy(out=hT[:], in_=pt[:])
                    q2 = wpool.tile([HID_F, P], BF16, tag="q2")
                    nc.vector.tensor_tensor(out=q2[:], in0=hT[:], in1=hT[:],
                                            op=mybir.AluOpType.mult)
                    q3 = wpool.tile([HID_F, P], BF16, tag="q3")
                    nc.vector.tensor_tensor(out=q3[:], in0=q2[:], in1=hT[:],
                                            op=mybir.AluOpType.mult)
                    ps2 = ppool.tile([P, OUT_F], F32, tag="k2")
                    nc.tensor.matmul(out=ps2[:], lhsT=hT[:], rhs=c_w2[0][:],
                                     start=True, stop=False)
                    nc.tensor.matmul(out=ps2[:], lhsT=q2[:], rhs=c_w2[1][:],
                                     start=False, stop=False)
                    nc.tensor.matmul(out=ps2[:], lhsT=q3[:], rhs=c_w2[2][:],
                                     start=False, stop=True)
                    hb2 = wpool.tile([P, OUT_F], F32, tag="hb2")
                    nc.vector.tensor_tensor(out=hb2[:], in0=ps2[:], in1=c_b2[:],
                                            op=mybir.AluOpType.add)
                    h2t = wpool.tile([P, OUT_F], BF16, tag="h2t")
                    nc.scalar.activation(out=h2t[:], in_=hb2[:],
                                         func=mybir.ActivationFunctionType.Copy,
                                         scale=c_mask[:, g:g + 1])
                    nc.sync.dma_start(out=h2_in[g * P:(g + 1) * P, :], in_=h2t[:])

                # ---------------- AllGather h2 ----------------
                if not skip_coll:
                    nc.gpsimd.collective_compute(
                        "AllGather", mybir.AluOpType.bypass,
                        replica_groups=[list(range(K))],
                        ins=[h2_in[:, :]], outs=[h2_tbl[:, :]],
                    )

                # ---------------- phase 3: aggregate + log_softmax ----------------
                for g in range(G):
                    D = int(Dg[g])
                    off = int(offs[g])
                    sts = []
                    for d in range(D if not skip_gather else 0):
                        st = spool.tile([P, OUT_F], BF16, tag=f"s3_{d}")
                        inst = nc.gpsimd.indirect_dma_start(
                            out=st[:, :],
                            out_offset=None,
                            in_=h2_tbl[:, :],
                            in_offset=bass.IndirectOffsetOnAxis(
                                ap=c_idx[:, off + d:off + d + 1], axis=0),
                        )
                        inst.ins.queue = QNAMES[(off + d) % 4]
                        sts.append(st)
                    tn = wpool.tile([P, OUT_F], F32, tag="tn")
                    if not sts:
                        nc.vector.memset(tn[:], 0.0)
                    elif len(sts) == 1:
                        nc.scalar.copy(out=tn[:], in_=sts[0][:])
                    else:
                        nc.vector.tensor_tensor(out=tn[:], in0=sts[0][:],
                                                in1=sts[1][:],
                                                op=mybir.AluOpType.add)
                        for d in range(2, len(sts)):
                            nc.vector.tensor_tensor(out=tn[:], in0=tn[:],
                                                    in1=sts[d][:],
                                                    op=mybir.AluOpType.add)
                    nc.vector.tensor_scalar_mul(tn[:], tn[:], c_dr[:, g:g + 1])
                    mx = wpool.tile([P, 1], F32, tag="mx")
                    nc.vector.tensor_reduce(out=mx[:], in_=tn[:],
                                            axis=mybir.AxisListType.X,
                                            op=mybir.AluOpType.max)
                    nmx = wpool.tile([P, 1], F32, tag="nmx")
                    nc.vector.tensor_scalar_mul(nmx[:], mx[:], -1.0)
                    et = wpool.tile([P, OUT_F], F32, tag="et")
                    se = wpool.tile([P, 1], F32, tag="se")
                    nc.scalar.activation(out=et[:], in_=tn[:],
                                         func=mybir.ActivationFunctionType.Exp,
                                         bias=nmx[:, :1], scale=1.0,
                                         accum_out=se[:, :1])
                    lse = wpool.tile([P, 1], F32, tag="lse")
                    nc.scalar.activation(out=lse[:], in_=se[:],
                                         func=mybir.ActivationFunctionType.Ln)
                    ot = wpool.tile([P, OUT_F], F32, tag="ot")
                    nc.vector.tensor_scalar(ot[:], tn[:], nmx[:, :1], lse[:, :1],
                                            mybir.AluOpType.add,
                                            mybir.AluOpType.subtract)
                    nc.sync.dma_start(out=y[g * P:(g + 1) * P, :], in_=ot[:])

    nc.compile()
    return nc


def kernel(x, edge_index, w1, b1, c1, w2, b2, c2):
    in_maps, meta = _host_prep(x, edge_index, w1, b1, c1, w2, b2, c2)
    nc = build_program(meta["Dg"], meta["offs"], meta["S"])
    res = run_bass_kernel_spmd(nc, in_maps, core_ids=list(range(K)))
    order = meta["order"]
    out = np.empty((N_NODES, OUT_F), dtype=np.float32)
    jr = np.arange(JREAL)
    for k in range(K):
        out[order[jr * K + k]] = res.results[k]["y"][:JREAL]
    return out



# revision 3
# speedup vs baseline: 2.8107x; 2.8107x over previous
"""KAN-GNN message passing on 8 TRN2 NeuronCores.

Strategy (data-parallel over nodes, per sharding hint):
 - Nodes ranked by in-degree, dealt round-robin to 8 cores (12544 local
   targets each, 98 windows of 128). Table position of node = core*J + j;
   the all-gathered tables are split into 4 chunks of 25088 rows so the
   hardware dma_gather's int16 indices can address them.
 - Phase 1: KAN layer 1 per local node (3 fused matmuls, bias+relu on DVE),
   AllGather of the bf16 h1 table.
 - Aggregation: edge-major. Per core, in-edges are sorted by (target
   window, source chunk); each (w,c) run is padded to a multiple of 128
   (shared across cores for SPMD) and fetched with one hardware dma_gather
   per <=1024-edge piece (SWDGE descriptor generation at ~0.34ns/row vs
   ~1.3us/128 rows for per-slot indirect DMAs). A per-sub-batch 0/1
   "binning" matrix B (fp8, streamed from DRAM) scatters the 128 gathered
   messages into the window's 128 targets via one PE matmul, accumulating
   in PSUM; drain applies 1/deg on DVE.
 - KAN layer 2 per window (PE transpose, DVE powers, 3 matmuls), h2 stored
   f32 (256B rows) and AllGathered; second edge-major gather + binning
   pass; log_softmax with deferred Ln to avoid ACT table thrash.
 - All indices/B matrices precomputed on host; per-core data as inputs.
"""
import numpy as np
import ml_dtypes

import concourse.bacc as bacc
import concourse.mybir as mybir
import concourse.tile as tile
import concourse.bass as bass
from concourse.bass_utils import run_bass_kernel_spmd

N_NODES = 100000
N_EDGES = 1600000
IN_F, HID_F, OUT_F = 128, 128, 64
K = 8               # cores
P = 128             # partitions
J = 12544           # local nodes per core (98*128), 12500 real + 44 pad
G = J // P          # 98 windows
JREAL = N_NODES // K
TBL = K * J         # 100352 rows in the all-gathered tables
NCH = 4             # chunks of the table (int16 index range)
CH = TBL // NCH     # 25088 rows per chunk
RMAX = 8            # max sub-batches (of 128 edges) per gather piece

BF16 = mybir.dt.bfloat16
F32 = mybir.dt.float32
I16 = mybir.dt.int16
FP8 = mybir.dt.float8e4


def _host_prep(x, edge_index, w1, b1, c1, w2, b2, c2):
    src = np.asarray(edge_index[0], dtype=np.int64)
    tgt = np.asarray(edge_index[1], dtype=np.int64)
    x = np.asarray(x, dtype=np.float32)

    deg = np.bincount(tgt, minlength=N_NODES)
    order = np.argsort(-deg, kind="stable")
    rank_of = np.empty(N_NODES, dtype=np.int64)
    rank_of[order] = np.arange(N_NODES)
    core_of = rank_of % K
    j_of = rank_of // K
    pos_of = core_of * J + j_of

    # per-edge attributes
    ek = core_of[tgt]                    # owning core (by target)
    we = j_of[tgt] // P                  # target window
    ce = core_of[src] // 2               # source chunk (= pos_of // CH)
    rel = pos_of[src] % CH               # in-chunk table row
    tcol = j_of[tgt] % P                 # target column within window

    key = (ek * G + we) * NCH + ce
    eorder = np.lexsort((rel, key))
    skey = key[eorder]
    counts = np.bincount(key, minlength=K * G * NCH).reshape(K, G, NCH)
    Lmax = counts.max(axis=0)                        # [G, NCH]
    Lpad = ((Lmax + P - 1) // P) * P                 # shared padded run lens

    run_off = np.zeros((G, NCH), dtype=np.int64)
    TOT = 0
    for w in range(G):
        for c in range(NCH):
            run_off[w, c] = TOT
            TOT += int(Lpad[w, c])
    SBTOT = TOT // P

    # place each core's sorted edges into the shared padded stream
    flat_counts = counts.reshape(-1)
    run_starts_e = np.concatenate([[0], np.cumsum(flat_counts)[:-1]])
    d_in_run = np.arange(len(skey)) - np.repeat(run_starts_e, flat_counts)
    kk = skey // (G * NCH)
    ww = (skey // NCH) % G
    cc = skey % NCH
    ppos = run_off[ww, cc] + d_in_run

    idx_rel = np.zeros((K, TOT), dtype=np.int16)     # pads -> row 0 (B=0)
    tcol_pad = np.full((K, TOT), -1, dtype=np.int64)
    idx_rel[kk, ppos] = rel[eorder].astype(np.int16)
    tcol_pad[kk, ppos] = tcol[eorder]

    # int16 index stream wrapped in 16 partitions, replicated x8 gpsimd cores
    blk = idx_rel.reshape(K, TOT // 16, 16).transpose(0, 2, 1)
    idx16 = np.ascontiguousarray(np.tile(blk, (1, 8, 1)))  # [K, 128, TOT//16]

    # binning matrices: Ball[k, p, s, t] = 1 if edge (s*128+p) targets t
    bm = np.zeros((K, TOT, P), dtype=ml_dtypes.float8_e4m3)
    kidx, eidx = np.nonzero(tcol_pad >= 0)
    bm[kidx, eidx, tcol_pad[kidx, eidx]] = 1.0
    Ball = np.ascontiguousarray(
        bm.reshape(K, SBTOT, P, P).transpose(0, 2, 1, 3))  # [K,128,SBTOT,128]

    # 1/deg per local target [K, P, G] (0 for pad targets)
    degs_kj = np.zeros((K, J), dtype=np.int64)
    degs_kj[core_of, j_of] = deg
    dr = 1.0 / np.maximum(degs_kj, 1).astype(np.float32)
    real = np.zeros((K, J), dtype=np.float32)
    real[:, :JREAL] = 1.0
    degrecip = np.ascontiguousarray(
        (dr * real).reshape(K, G, P).transpose(0, 2, 1))

    # xT shards, bf16 [K][IN_F, J]
    xT = np.zeros((K, IN_F, J), dtype=ml_dtypes.bfloat16)
    for k in range(K):
        nodes_k = order[np.arange(JREAL) * K + k]
        xT[k, :, :JREAL] = x[nodes_k].T.astype(ml_dtypes.bfloat16)

    # fused KAN weights
    A1 = (w1 + 0.1 * c1[:, :, 0]).astype(ml_dtypes.bfloat16)
    B1 = (0.1 * c1[:, :, 1]).astype(ml_dtypes.bfloat16)
    C1 = (0.1 * c1[:, :, 2]).astype(ml_dtypes.bfloat16)
    A2 = (w2 + 0.1 * c2[:, :, 0]).astype(ml_dtypes.bfloat16)
    B2 = (0.1 * c2[:, :, 1]).astype(ml_dtypes.bfloat16)
    C2 = (0.1 * c2[:, :, 2]).astype(ml_dtypes.bfloat16)
    b1b = np.tile(np.asarray(b1, np.float32)[None, :], (P, 1))
    b2b = np.tile(np.asarray(b2, np.float32)[None, :], (P, 1))
    ident = np.eye(P, dtype=np.float32)

    in_maps = []
    for k in range(K):
        in_maps.append({
            "xT": xT[k],
            "idx16": idx16[k],
            "Ball": Ball[k],
            "degrecip": degrecip[k],
            "A1": A1, "B1": B1, "C1": C1,
            "A2": A2, "B2": B2, "C2": C2,
            "b1b": b1b, "b2b": b2b, "ident": ident,
        })
    meta = {"Lpad": Lpad, "run_off": run_off, "TOT": TOT, "SBTOT": SBTOT,
            "order": order}
    return in_maps, meta


def build_program(meta):
    Lpad = meta["Lpad"]
    run_off = meta["run_off"]
    TOT = int(meta["TOT"])
    SBTOT = int(meta["SBTOT"])
    TOTC = TOT // 16

    nc = bacc.Bacc("TRN2", target_bir_lowering=False, debug=False, num_devices=K,
                   num_swdge_queues=4)

    xT = nc.dram_tensor("xT", [IN_F, J], BF16, kind="ExternalInput")
    idx16 = nc.dram_tensor("idx16", [P, TOTC], I16, kind="ExternalInput")
    Ball = nc.dram_tensor("Ball", [P, SBTOT, P], FP8, kind="ExternalInput")
    degrecip = nc.dram_tensor("degrecip", [P, G], F32, kind="ExternalInput")
    A1 = nc.dram_tensor("A1", [IN_F, HID_F], BF16, kind="ExternalInput")
    B1 = nc.dram_tensor("B1", [IN_F, HID_F], BF16, kind="ExternalInput")
    C1 = nc.dram_tensor("C1", [IN_F, HID_F], BF16, kind="ExternalInput")
    A2 = nc.dram_tensor("A2", [HID_F, OUT_F], BF16, kind="ExternalInput")
    B2 = nc.dram_tensor("B2", [HID_F, OUT_F], BF16, kind="ExternalInput")
    C2 = nc.dram_tensor("C2", [HID_F, OUT_F], BF16, kind="ExternalInput")
    b1b = nc.dram_tensor("b1b", [P, HID_F], F32, kind="ExternalInput")
    b2b = nc.dram_tensor("b2b", [P, OUT_F], F32, kind="ExternalInput")
    ident = nc.dram_tensor("ident", [P, P], F32, kind="ExternalInput")
    y = nc.dram_tensor("y", [J, OUT_F], F32, kind="ExternalOutput")

    h1_in = nc.dram_tensor("h1_in", [J, HID_F], BF16, kind="Internal")
    h1_tbl = nc.dram_tensor("h1_tbl", [TBL, HID_F], BF16, kind="Internal",
                            addr_space="Shared")
    h2_in = nc.dram_tensor("h2_in", [J, OUT_F], F32, kind="Internal")
    h2_tbl = nc.dram_tensor("h2_tbl", [TBL, OUT_F], F32, kind="Internal",
                            addr_space="Shared")

    # pieces per window: (chunk, padded-stream offset, n)
    pieces_w = []
    for w in range(G):
        pieces = []
        for c in range(NCH):
            L = int(Lpad[w, c])
            off = int(run_off[w, c])
            o = 0
            while o < L:
                n = min(L - o, RMAX * P)
                pieces.append((c, off + o, n))
                o += n
        pieces_w.append(pieces)

    qctr = [0]

    def next_q():
        q = qctr[0] % 4
        qctr[0] += 1
        return q

    with tile.TileContext(nc) as tc:
        with (
            tc.tile_pool(name="consts", bufs=1) as cpool,
            tc.tile_pool(name="work", bufs=3) as wpool,
            tc.tile_pool(name="gath", bufs=6) as gpool,
            tc.tile_pool(name="bmat", bufs=6) as bpool,
            tc.tile_pool(name="psum", bufs=2, space="PSUM") as ppool,
        ):
            c_idx = cpool.tile([P, TOTC], I16, tag="idx16")
            nc.sync.dma_start(out=c_idx[:], in_=idx16[:, :])
            c_dr = cpool.tile([P, G], F32, tag="dr")
            nc.sync.dma_start(out=c_dr[:], in_=degrecip[:, :])
            c_w1 = []
            for nm, t in (("A1", A1), ("B1", B1), ("C1", C1)):
                wt = cpool.tile([IN_F, HID_F], BF16, tag=nm)
                nc.sync.dma_start(out=wt[:], in_=t[:, :])
                c_w1.append(wt)
            c_w2 = []
            for nm, t in (("A2", A2), ("B2", B2), ("C2", C2)):
                wt = cpool.tile([HID_F, OUT_F], BF16, tag=nm)
                nc.sync.dma_start(out=wt[:], in_=t[:, :])
                c_w2.append(wt)
            c_b1 = cpool.tile([P, HID_F], F32, tag="b1b")
            nc.sync.dma_start(out=c_b1[:], in_=b1b[:, :])
            c_b2 = cpool.tile([P, OUT_F], F32, tag="b2b")
            nc.sync.dma_start(out=c_b2[:], in_=b2b[:, :])
            c_id = cpool.tile([P, P], F32, tag="ident")
            nc.sync.dma_start(out=c_id[:], in_=ident[:, :])
            # per-window softmax state, filled in phase 3
            tn_all = cpool.tile([P, G, OUT_F], F32, tag="tn_all")
            nmx_all = cpool.tile([P, G], F32, tag="nmx_all")
            se_all = cpool.tile([P, G], F32, tag="se_all")

            # ---------------- phase 1: KAN layer 1 on the shard ----------
            for g in range(G):
                xt = wpool.tile([IN_F, P], BF16, tag="xt")
                nc.sync.dma_start(out=xt[:], in_=xT[:, g * P:(g + 1) * P])
                x2 = wpool.tile([IN_F, P], BF16, tag="x2")
                nc.vector.tensor_tensor(out=x2[:], in0=xt[:], in1=xt[:],
                                        op=mybir.AluOpType.mult)
                x3 = wpool.tile([IN_F, P], BF16, tag="x3")
                nc.vector.tensor_tensor(out=x3[:], in0=x2[:], in1=xt[:],
                                        op=mybir.AluOpType.mult)
                ps = ppool.tile([P, HID_F], F32, tag="big")
                nc.tensor.matmul(out=ps[:], lhsT=xt[:], rhs=c_w1[0][:],
                                 start=True, stop=False)
                nc.tensor.matmul(out=ps[:], lhsT=x2[:], rhs=c_w1[1][:],
                                 start=False, stop=False)
                nc.tensor.matmul(out=ps[:], lhsT=x3[:], rhs=c_w1[2][:],
                                 start=False, stop=True)
                hb = wpool.tile([P, HID_F], F32, tag="hb")
                nc.vector.tensor_tensor(out=hb[:], in0=ps[:], in1=c_b1[:],
                                        op=mybir.AluOpType.add)
                h1t = wpool.tile([P, HID_F], BF16, tag="h1t")
                nc.vector.tensor_scalar_max(h1t[:], hb[:], 0.0)
                nc.sync.dma_start(out=h1_in[g * P:(g + 1) * P, :], in_=h1t[:])

            # ---------------- AllGather h1 -------------------------------
            nc.gpsimd.collective_compute(
                "AllGather", mybir.AluOpType.bypass,
                replica_groups=[list(range(K))],
                ins=[h1_in[:, :]], outs=[h1_tbl[:, :]],
            )

            # ---------------- phase 2: aggregate + KAN layer 2 -----------
            for w in range(G):
                pieces = pieces_w[w]
                pbin = ppool.tile([P, HID_F], F32, tag="big")
                if not pieces:
                    agg = wpool.tile([P, HID_F], F32, tag="agg")
                    nc.vector.memset(agg[:], 0.0)
                else:
                    nsb_tot = sum(n // P for (_, _, n) in pieces)
                    si = 0
                    for (c, poff, n) in pieces:
                        nsb = n // P
                        gt = gpool.tile([P, RMAX, HID_F], BF16, tag="g1")
                        nc.gpsimd.dma_gather(
                            gt[:, :nsb, :],
                            h1_tbl[c * CH:(c + 1) * CH, :],
                            c_idx[:, poff // 16:(poff + n) // 16],
                            n, n, HID_F, queue_num=next_q())
                        bt = bpool.tile([P, RMAX, P], FP8, tag="b")
                        nc.sync.dma_start(
                            out=bt[:, :nsb, :],
                            in_=Ball[:, poff // P:(poff + n) // P, :])
                        for s in range(nsb):
                            nc.tensor.matmul(
                                out=pbin[:], lhsT=bt[:, s, :], rhs=gt[:, s, :],
                                start=(si == 0), stop=(si == nsb_tot - 1))
                            si += 1
                    agg = wpool.tile([P, HID_F], F32, tag="agg")
                    nc.vector.tensor_scalar_mul(agg[:], pbin[:],
                                                c_dr[:, w:w + 1])
                pt = ppool.tile([P, P], F32, tag="tr")
                nc.tensor.transpose(out=pt[:], in_=agg[:], identity=c_id[:])
                hT = wpool.tile([HID_F, P], BF16, tag="hT")
                nc.vector.tensor_scalar_mul(hT[:], pt[:], 1.0)
                q2 = wpool.tile([HID_F, P], BF16, tag="q2")
                nc.vector.tensor_tensor(out=q2[:], in0=hT[:], in1=hT[:],
                                        op=mybir.AluOpType.mult)
                q3 = wpool.tile([HID_F, P], BF16, tag="q3")
                nc.vector.tensor_tensor(out=q3[:], in0=q2[:], in1=hT[:],
                                        op=mybir.AluOpType.mult)
                ps2 = ppool.tile([P, OUT_F], F32, tag="small")
                nc.tensor.matmul(out=ps2[:], lhsT=hT[:], rhs=c_w2[0][:],
                                 start=True, stop=False)
                nc.tensor.matmul(out=ps2[:], lhsT=q2[:], rhs=c_w2[1][:],
                                 start=False, stop=False)
                nc.tensor.matmul(out=ps2[:], lhsT=q3[:], rhs=c_w2[2][:],
                                 start=False, stop=True)
                hb2 = wpool.tile([P, OUT_F], F32, tag="hb2")
                nc.vector.tensor_tensor(out=hb2[:], in0=ps2[:], in1=c_b2[:],
                                        op=mybir.AluOpType.add)
                nc.sync.dma_start(out=h2_in[w * P:(w + 1) * P, :], in_=hb2[:])

            # ---------------- AllGather h2 -------------------------------
            nc.gpsimd.collective_compute(
                "AllGather", mybir.AluOpType.bypass,
                replica_groups=[list(range(K))],
                ins=[h2_in[:, :]], outs=[h2_tbl[:, :]],
            )

            # ---------------- phase 3: aggregate + log_softmax -----------
            for w in range(G):
                pieces = pieces_w[w]
                pb3 = ppool.tile([P, OUT_F], F32, tag="small")
                if not pieces:
                    nc.vector.memset(tn_all[:, w, :], 0.0)
                else:
                    nsb_tot = sum(n // P for (_, _, n) in pieces)
                    si = 0
                    for (c, poff, n) in pieces:
                        nsb = n // P
                        gt = gpool.tile([P, RMAX, OUT_F], F32, tag="g2")
                        nc.gpsimd.dma_gather(
                            gt[:, :nsb, :],
                            h2_tbl[c * CH:(c + 1) * CH, :],
                            c_idx[:, poff // 16:(poff + n) // 16],
                            n, n, OUT_F, queue_num=next_q())
                        m16 = gpool.tile([P, RMAX, OUT_F], BF16, tag="m16")
                        nc.vector.tensor_scalar_mul(m16[:, :nsb, :],
                                                    gt[:, :nsb, :], 1.0)
                        bt = bpool.tile([P, RMAX, P], FP8, tag="b")
                        nc.sync.dma_start(
                            out=bt[:, :nsb, :],
                            in_=Ball[:, poff // P:(poff + n) // P, :])
                        for s in range(nsb):
                            nc.tensor.matmul(
                                out=pb3[:], lhsT=bt[:, s, :],
                                rhs=m16[:, s, :],
                                start=(si == 0), stop=(si == nsb_tot - 1))
                            si += 1
                    nc.vector.tensor_scalar_mul(tn_all[:, w, :], pb3[:],
                                                c_dr[:, w:w + 1])
                mx = wpool.tile([P, 1], F32, tag="mx")
                nc.vector.tensor_reduce(out=mx[:], in_=tn_all[:, w, :],
                                        axis=mybir.AxisListType.X,
                                        op=mybir.AluOpType.max)
                nc.vector.tensor_scalar_mul(nmx_all[:, w:w + 1], mx[:], -1.0)
                et = wpool.tile([P, OUT_F], F32, tag="et")
                nc.scalar.activation(out=et[:], in_=tn_all[:, w, :],
                                     func=mybir.ActivationFunctionType.Exp,
                                     bias=nmx_all[:, w:w + 1], scale=1.0,
                                     accum_out=se_all[:, w:w + 1])
            lse_all = cpool.tile([P, G], F32, tag="lse_all")
            nc.scalar.activation(out=lse_all[:], in_=se_all[:],
                                 func=mybir.ActivationFunctionType.Ln)
            for w in range(G):
                ot = wpool.tile([P, OUT_F], F32, tag="ot")
                nc.vector.tensor_scalar(ot[:], tn_all[:, w, :],
                                        nmx_all[:, w:w + 1],
                                        lse_all[:, w:w + 1],
                                        mybir.AluOpType.add,
                                        mybir.AluOpType.subtract)
                nc.sync.dma_start(out=y[w * P:(w + 1) * P, :], in_=ot[:])

    nc.compile()
    return nc


def kernel(x, edge_index, w1, b1, c1, w2, b2, c2):
    in_maps, meta = _host_prep(x, edge_index, w1, b1, c1, w2, b2, c2)
    nc = build_program(meta)
    res = run_bass_kernel_spmd(nc, in_maps, core_ids=list(range(K)))
    order = meta["order"]
    out = np.empty((N_NODES, OUT_F), dtype=np.float32)
    jr = np.arange(JREAL)
    for k in range(K):
        out[order[jr * K + k]] = res.results[k]["y"][:JREAL]
    return out
